# revision 1
# baseline (speedup 1.0000x reference)
"""Trainium2 Bass kernel for nn_Contour_to_distance_map.

Math (per polygon p, mesh pixel m=(mx,my), vertex k, with u=c_k-m, v=c_{k+1}-m):
  nd=|u|, nr=|v|, cross = u_y v_x - u_x v_y, dot = u.v
  ang = arccos(clip(dot/(nd nr), -1+eps, 1-eps))
      = pi/2 - 2*arctan(clip(u_half, -U, U)),  u_half = dot/(nd*nr + |cross|)
  (Lagrange: (nd*nr)^2 = cross^2 + dot^2 = X; X is also Q1_k*Q1_{k+1} with
   Q1 = nd^2, whose outer-product expansion is a sum of nonnegatives -> no
   cancellation.)
  winding = |sum_k tanh(1e5*cross)*ang|; out = winding*min_k nd / max(...)

Every per-(pixel,k) field is an outer sum P_k(i) + v_k(j) over row/col
coordinates, so the device evaluates tiny-contraction bf16-3-split matmuls
(exact fp32 reconstruction) plus elementwise passes. Data-parallel over 8
cores: core c -> polygon c//2, row-half c%2. Global-max normalization on host
(ratio is scale-invariant; the 1/2pi cancels).
"""

import numpy as np
import ml_dtypes

import concourse.bass as bass
import concourse.bacc as bacc
import concourse.tile as tile
import concourse.mybir as mybir
import concourse.bass_utils as bass_utils
import concourse.dve_ops as dve_ops
from concourse.dve_ops import AFFINE_MUL_REDUCE, DveOp
from concourse.dve_spec import (Spec, Src0, Src1, C0, C1, Zero, maxx, minn,
                                lower, _has_src1)
from concourse.dve_uop import DveOpSpec
from concourse.tile_rust import add_dep_helper

F32 = mybir.dt.float32
BF16 = mybir.dt.bfloat16
I32 = mybir.dt.int32

SIZE = 256
K = 64
NPAIR = K // 2          # 32 vertex pairs
# PE row-group layout per pair (each block in its own 32-row array group so
# the four matmuls run concurrently on different sub-arrays):
#   rows [ 0:12)  cross (6 bf16-split rows per k)
#   rows [32:44)  dot   (6 per k)
#   rows [64:88)  X     (12 per k: A3 + B3 + {hh,hm,mh} products)
#   rows [96:104) Q1    (4 per k: 2-split)
NROWS = 104
HALF_PAIRS = NPAIR // 2
HCOLS = HALF_PAIRS * 512       # 8192 real elements per half per quantity
EPS = 1e-5
K_SIGN = 100000.0
U_CLIP = float(np.tan(np.arcsin(1.0 - EPS) / 2.0))   # ~0.9955378
MINACC_INIT = 3.0e38

_BF = ml_dtypes.bfloat16


# ---------------- custom fused DVE ops ---------------- #

def _make_op(name, spec):
    """Author + register a custom DVE op at runtime (sha computed here)."""
    for op in dve_ops.OPS:
        if op.name == name:
            return op
    row = dve_ops._CUSTOM_DVE_ROW_BASE + len(dve_ops.OPS)
    assert row < 0x20
    dve_ops._SUB_OPCODE_FOR_NAME[name] = row
    shas = {}
    for ver in ("v3", "v4"):
        try:
            s = DveOpSpec(name=name, opcode=row, uops=lower(spec, ver=ver),
                          rd1_en=_has_src1(spec))
            shas[ver] = s.sha(ver)
        except Exception:
            pass
    op = DveOp(name, spec, subdim=False, uops_sha=shas)
    dve_ops.OPS.append(op)
    dve_ops.CUSTOM_DVE_SPECS[name] = spec
    return op


# g = |in0| + in1
ABS_ADD_ANT = _make_op("ABS_ADD_ANT", Spec(
    body=maxx(Src0, Zero - Src0) + Src1,
    reference=lambda in0, in1, s0, s1, imm2:
        np.abs(in0.astype(np.float32)) + in1,
))

# uc = clip(in0*in1, C1, C0)  (pass s0=+U, s1=-U)
MUL_CLIP_ANT = _make_op("MUL_CLIP_ANT", Spec(
    body=minn(maxx(Src0 * Src1, C1), C0),
    reference=lambda in0, in1, s0, s1, imm2:
        np.minimum(np.maximum(in0.astype(np.float32) * in1, s1), s0),
))


# ---------------- host-side coefficients ---------------- #

def _split3(x):
    """f64 -> three bf16 planes summing to ~fp32 precision."""
    h = np.asarray(x, _BF).astype(np.float64)
    m = np.asarray(x - h, _BF).astype(np.float64)
    l = np.asarray(x - h - m, _BF).astype(np.float64)
    return (h.astype(_BF), m.astype(_BF), l.astype(_BF))


def _core_coeffs(C, core):
    """lhsT (NROWS, NPAIR*128) + rhs (NROWS, NPAIR*2048) bf16 for one core."""
    p, hh = core // 2, core % 2
    mx = (hh * 128 + np.arange(128, dtype=np.float64)) / SIZE
    my = np.arange(SIZE, dtype=np.float64) / SIZE
    cx, cy = C[p, :, 0], C[p, :, 1]
    c1x, c1y = np.roll(cx, -1), np.roll(cy, -1)
    ex, ey = c1x - cx, c1y - cy

    P1 = (cx[None, :] - mx[:, None]) ** 2
    v1 = (cy[None, :] - my[:, None]) ** 2
    P1n = np.roll(P1, -1, axis=1)
    v1n = np.roll(v1, -1, axis=1)
    A = P1 * P1n
    B = v1 * v1n
    P3 = ey[None, :] * mx[:, None] + (cy * ex - cx * ey)[None, :]
    v3 = -ex[None, :] * my[:, None]
    P4 = (cx[None, :] - mx[:, None]) * (c1x[None, :] - mx[:, None])
    v4 = (cy[None, :] - my[:, None]) * (c1y[None, :] - my[:, None])

    sp = {}
    for name, arr in [("P1", P1), ("v1", v1), ("P1n", P1n), ("v1n", v1n),
                      ("A", A), ("B", B), ("P3", P3), ("v3", v3),
                      ("P4", P4), ("v4", v4)]:
        sp[name] = _split3(arr)

    ones_i = np.ones(128, _BF)
    ones_j = np.ones(SIZE, _BF)
    PRODS = [(0, 0), (0, 1), (1, 0)]   # hh, hm, mh split products

    def block_rows(k, blk):
        rows = []
        if blk == 0:    # cross = P3 + v3
            for t in range(3):
                rows.append((sp["P3"][t][:, k], ones_j))
            for t in range(3):
                rows.append((ones_i, sp["v3"][t][:, k]))
        elif blk == 1:  # dot = P4 + v4
            for t in range(3):
                rows.append((sp["P4"][t][:, k], ones_j))
            for t in range(3):
                rows.append((ones_i, sp["v4"][t][:, k]))
        elif blk == 2:  # X = A + B + P1*v1n + P1n*v1 (all nonneg groups)
            for t in range(3):
                rows.append((sp["A"][t][:, k], ones_j))
            for t in range(3):
                rows.append((ones_i, sp["B"][t][:, k]))
            for a, b in PRODS:
                rows.append((sp["P1"][a][:, k], sp["v1n"][b][:, k]))
            for a, b in PRODS:
                rows.append((sp["P1n"][a][:, k], sp["v1"][b][:, k]))
        else:           # Q1 = P1 + v1 (2-split)
            for t in range(2):
                rows.append((sp["P1"][t][:, k], ones_j))
            for t in range(2):
                rows.append((ones_i, sp["v1"][t][:, k]))
        return rows

    BLK_BASE = [0, 32, 64, 96]
    BLK_KROWS = [6, 6, 12, 4]

    lhsT = np.zeros((NROWS, NPAIR, 128), _BF)
    rhs = np.zeros((NROWS, NPAIR, 512), _BF)
    for pp in range(NPAIR):
        for t in range(2):
            k = 2 * pp + t
            for blk in range(4):
                rows = block_rows(k, blk)
                base = BLK_BASE[blk] + t * BLK_KROWS[blk]
                for r, (li, rj) in enumerate(rows):
                    lhsT[base + r, pp, :] = li
                    rhs[base + r, pp, t * 256:(t + 1) * 256] = rj
    return lhsT.reshape(NROWS, -1), rhs.reshape(NROWS, -1)


_PROGRAM = None


def _build_program():
    nc = bacc.Bacc("TRN2", target_bir_lowering=False, debug=False,
                   enable_asserts=False, num_devices=1)
    lhsT_d = nc.dram_tensor("lhsT", [NROWS, NPAIR * 128], BF16,
                            kind="ExternalInput").ap()
    rhs_d = nc.dram_tensor("rhs", [NROWS, NPAIR * 512], BF16,
                           kind="ExternalInput").ap()
    out_d = nc.dram_tensor("pm2", [128, SIZE], F32, kind="ExternalOutput").ap()

    AF = mybir.ActivationFunctionType
    ALU = mybir.AluOpType
    with tile.TileContext(nc, pool_alloc_mode="queue") as tc:
        with tc.tile_pool(name="lhsp", bufs=1) as lhsp, \
             tc.tile_pool(name="rhsp", bufs=3) as rhsp, \
             tc.tile_pool(name="fields", bufs=1) as fieldp, \
             tc.tile_pool(name="fin", bufs=1) as finp, \
             tc.tile_pool(name="ps", bufs=2, space="PSUM") as psp, \
             tc.tile_pool(name="q1ps", bufs=1, space="PSUM") as q1psp:

            lhsT_sb = lhsp.tile([NROWS, NPAIR * 128], BF16)
            # first chunk on the sync queue ahead of the rhs stream (pair 0
            # needs it); the rest in parallel on the gpsimd queue
            NL = 4
            lw = NPAIR * 128 // NL
            nc.sync.dma_start(lhsT_sb[:, 0:lw], lhsT_d[:, 0:lw])
            for c in range(1, NL):
                nc.gpsimd.dma_start(lhsT_sb[:, c * lw:(c + 1) * lw],
                                    lhsT_d[:, c * lw:(c + 1) * lw])

            minacc = finp.tile([128, 1024], F32)
            nc.vector.memset(minacc[:, :], MINACC_INIT)

            wparts = []
            prev_act = None  # last ACT inst of previous set-phase
            for half in range(2):
                # cdf: interleaved [cross(512)|dot(512)] blocks per pair
                cdf = fieldp.tile([128, HALF_PAIRS * 1024], F32, tag="cdf")
                denf = fieldp.tile([128, HCOLS], F32, tag="denf")
                sf = fieldp.tile([128, HCOLS], F32, tag="sf")
                af = fieldp.tile([128, HCOLS], F32, tag="af")

                # ---- streaming: PE matmuls + psum evacuation [sqrt set] ----
                first_act = None
                last_act = None
                q1t = None
                for i in range(HALF_PAIRS):
                    pp = half * HALF_PAIRS + i
                    rhs_t = rhsp.tile([NROWS, 512], BF16, tag="rhs")
                    nc.sync.dma_start(rhs_t[:, :],
                                      rhs_d[:, pp * 512:(pp + 1) * 512])
                    ps = psp.tile([128, 1536], F32, tag="ps")
                    if i % 2 == 0:
                        q1t = q1psp.tile([128, 1024], F32, tag="q1")
                    lt = lhsT_sb[:, pp * 128:(pp + 1) * 128]
                    # four matmuls in distinct PE row-groups -> concurrent
                    nc.tensor.matmul(ps[:, 0:512], lt[0:12, :],
                                     rhs_t[0:12, :], start=True, stop=True)
                    nc.tensor.matmul(ps[:, 512:1024], lt[32:44, :],
                                     rhs_t[32:44, :], start=True, stop=True)
                    nc.tensor.matmul(ps[:, 1024:1536], lt[64:88, :],
                                     rhs_t[64:88, :], start=True, stop=True)
                    nc.tensor.matmul(q1t[:, (i % 2) * 512:(i % 2) * 512 + 512],
                                     lt[96:104, :], rhs_t[96:104, :],
                                     start=True, stop=True,
                                     tile_position=(96, 0))
                    i1 = nc.scalar.activation(cdf[:, i * 1024:(i + 1) * 1024],
                                              ps[:, 0:1024], AF.Copy)
                    i2 = nc.scalar.activation(denf[:, i * 512:(i + 1) * 512],
                                              ps[:, 1024:1536], AF.Sqrt)
                    if i % 2 == 1:
                        nc.vector.tensor_tensor(minacc[:, :], minacc[:, :],
                                                q1t[:, 0:1024], op=ALU.min)
                    if first_act is None:
                        first_act = i1
                    last_act = i2
                if prev_act is not None:
                    add_dep_helper(first_act.ins, prev_act.ins, sync=False,
                                   reason="ACT table-set phase order")
                prev_act = last_act

                # strided views: cross / dot halves of cdf
                def cview(ch, which, width):
                    lo = ch * width
                    v = cdf[:, lo * 2:(ch + 1) * width * 2]
                    v = v.rearrange("p (b q) -> p b q", q=1024)
                    return v[:, :, which * 512:(which + 1) * 512]

                # ---- C phase part 1 [tanh set]: overlaps the DVE-only
                # B phase below (reads only the cross views of cdf) ----
                NCH = 4
                CW = HCOLS // NCH  # 2048
                tanh_insts = []
                for ch in range(NCH):
                    sl = slice(ch * CW, (ch + 1) * CW)
                    it = nc.scalar.activation(sf[:, sl], cview(ch, 0, CW),
                                              AF.Tanh, scale=K_SIGN)
                    tanh_insts.append(it)
                add_dep_helper(tanh_insts[0].ins, prev_act.ins, sync=False,
                               reason="ACT table-set phase order")

                # ---- B phase (DVE only) ----
                for ch in range(NCH):
                    sl = slice(ch * CW, (ch + 1) * CW)
                    # g = |cross| + den   (in place over denf)
                    nc.vector._custom_dve(ABS_ADD_ANT, out=denf[:, sl],
                                          in0=cview(ch, 0, CW),
                                          in1=denf[:, sl])
                    # rg = 1/g
                    nc.vector.reciprocal_approx_fast(out=denf[:, sl],
                                                     in_=denf[:, sl])
                    # uc = clip(dot*rg)  (in place over cdf dot-blocks)
                    nc.vector._custom_dve(MUL_CLIP_ANT, out=cview(ch, 1, CW),
                                          in0=cview(ch, 1, CW),
                                          in1=denf[:, sl],
                                          s0=U_CLIP, s1=-U_CLIP)

                atan_insts = []
                for ch in range(NCH):
                    sl = slice(ch * CW, (ch + 1) * CW)
                    ia = nc.scalar.activation(af[:, sl], cview(ch, 1, CW),
                                              AF.Arctan)
                    atan_insts.append(ia)
                    # prod = (a*-2 + pi/2)*s  (in place over af)
                    nc.vector._custom_dve(AFFINE_MUL_REDUCE, out=af[:, sl],
                                          in0=af[:, sl], in1=sf[:, sl],
                                          s0=-2.0, s1=float(np.pi / 2))
                add_dep_helper(atan_insts[0].ins, tanh_insts[-1].ins,
                               sync=False, reason="tanh set before atan set")
                prev_act = atan_insts[-1]

                # ksum tournament fold -> wp (128, 512). Half 0's folds ride
                # the idle GpSimd (hidden under half 1's streaming).
                eng = nc.gpsimd if half == 0 else nc.vector
                eng.tensor_tensor(af[:, 0:2048], af[:, 0:2048],
                                  af[:, 2048:4096], op=ALU.add)
                eng.tensor_tensor(af[:, 4096:6144], af[:, 4096:6144],
                                  af[:, 6144:8192], op=ALU.add)
                eng.tensor_tensor(af[:, 0:2048], af[:, 0:2048],
                                  af[:, 4096:6144], op=ALU.add)
                eng.tensor_tensor(af[:, 0:1024], af[:, 0:1024],
                                  af[:, 1024:2048], op=ALU.add)
                wp = finp.tile([128, 512], F32, tag=f"wp{half}")
                eng.tensor_tensor(wp[:, :], af[:, 0:512], af[:, 512:1024],
                                  op=ALU.add)
                wparts.append(wp)

            # ---- finals (minq folds first: they only depend on the kmin
            # accumulator, so they overlap the C2 tail) ----
            minq = finp.tile([128, 256], F32)
            nc.vector.tensor_tensor(minq[:, :], minacc[:, 0:256],
                                    minacc[:, 256:512], op=ALU.min)
            nc.vector.tensor_tensor(minq[:, :], minq[:, :],
                                    minacc[:, 512:768], op=ALU.min)
            nc.vector.tensor_tensor(minq[:, :], minq[:, :],
                                    minacc[:, 768:1024], op=ALU.min)
            w = finp.tile([128, 512], F32)
            nc.vector.tensor_tensor(w[:, :], wparts[0][:, :], wparts[1][:, :],
                                    op=ALU.add)
            wsum = finp.tile([128, 256], F32)
            nc.vector.tensor_tensor(wsum[:, :], w[:, 0:256], w[:, 256:512],
                                    op=ALU.add)
            nc.vector.tensor_tensor(wsum[:, :], wsum[:, :], wsum[:, :],
                                    op=ALU.mult)
            nc.vector.tensor_tensor(wsum[:, :], wsum[:, :], minq[:, :],
                                    op=ALU.mult)
            nc.sync.dma_start(out_d[:, :], wsum[:, :])

    nc.compile()
    return nc


def _get_program():
    global _PROGRAM
    if _PROGRAM is None:
        _PROGRAM = _build_program()
    return _PROGRAM


def kernel(contour: np.ndarray) -> np.ndarray:
    contour = np.asarray(contour)
    b, n, k, _ = contour.shape
    assert (b, n, k) == (2, 2, K)
    C = contour.reshape(b * n, K, 2).astype(np.float64)

    nc = _get_program()
    in_maps = []
    for core in range(8):
        lhsT, rhs = _core_coeffs(C, core)
        in_maps.append({"lhsT": lhsT, "rhs": rhs})

    res = bass_utils.run_bass_kernel_spmd(nc, in_maps, core_ids=list(range(8)))

    pm2 = np.stack([res.results[c]["pm2"] for c in range(8)])  # (8,128,256)
    pm = np.sqrt(np.maximum(pm2.astype(np.float64), 0.0))
    dmap = (pm / pm.max()).astype(np.float32)
    out = np.zeros((b * n, SIZE, SIZE), np.float32)
    for core in range(8):
        p, hh = core // 2, core % 2
        out[p, hh * 128:(hh + 1) * 128, :] = dmap[core]
    return out.reshape(b, n, SIZE, SIZE)



# revision 5
# speedup vs baseline: 1.7179x; 1.7179x over previous
"""Trainium2 Bass kernel for nn_Contour_to_distance_map.

Reformulation: the reference winding |Σ_k tanh(1e5·cross_k)·arccos(clip(cos_k))|/2π
equals the integer ray-crossing count for all pixels outside hair-thin bands
around edge lines (validated: rel L2 vs reference 9.5e-3 << 2e-2 budget).

Per pixel m=(py,px) and edge a_k→b_k (py=coord0=partition/row, px=coord1=col):
  crossing contribution f_k = m_up·[cL>0] − m_dn·[cL<0]
    cL = v1x·(py−a1?) ... cL_k(i,j) = v1x·py_i − v1y·px_j + (v1y·ax − v1x·ay)
    m_up = [ay ≤ py < by], m_dn = [by ≤ py < ay]  (per-partition constants)
  w = Σ_k f_k = 0.5·Σ_k sign(M_k) + B(part)
    M_k = mask_k(i)·cL_k − 8·(1−mask_k(i))   (strictly negative when masked out)
    B(part) = 32 − Σ_k m_dn_k(part)
  out = |w| · min_k dist(m, a_k); device computes w²·min_k dist²; host sqrts
  and applies the global max normalization (scale-invariant).

All per-k fields are outer sums P_k(i)+v_k(j) → tiny-contraction bf16-2-split
matmuls. Engines: PE produces M and Q1=dist² psum tiles and accumulates the
sign-field sums via identity matmuls (exact: signs are ±1 in bf16, psum fp32);
ACT does Sign (psum→sbuf bf16; every table set has sign → no table thrash);
DVE does the running min over Q1. Data-parallel: core c → polygon c//2,
row-half c%2.
"""

import numpy as np
import ml_dtypes

import concourse.bass as bass
import concourse.bacc as bacc
import concourse.tile as tile
import concourse.mybir as mybir
import concourse.bass_utils as bass_utils

F32 = mybir.dt.float32
BF16 = mybir.dt.bfloat16

SIZE = 256
K = 64
NPAIR = K // 2           # 32 pairs of vertices, 2 k per pair
NGRP = NPAIR // 2        # 16 groups of 2 pairs (4 k)
# lhsT/rhs row layout (compact DRAM form, 18 rows):
#   rows [0:10)  M block: per k 5 rows: maskPMh, maskPMm, mask(vMh), mask(vMm),
#                -8(1-mask)
#   rows [10:18) Q block: per k 4 rows: PQh, PQm, vQh, vQm
# SBUF form is [40, ...]: M at partitions 0:10, Q at partitions 32:40 so the
# two matmuls sit in different PE row groups.
MROWS = 10
QROWS = 8
DROWS = MROWS + QROWS
MINACC_INIT = 3.0e38

_BF = ml_dtypes.bfloat16


def _split2(x):
    """f64 -> two bf16 planes summing to ~16-bit-mantissa precision."""
    h = np.asarray(x, _BF).astype(np.float64)
    m = np.asarray(x - h, _BF).astype(np.float64)
    return h.astype(_BF), m.astype(_BF)


def _core_coeffs(C, core):
    """Inputs for one core: lhsT (18, NPAIR*128), rhs (18, NPAIR*512) bf16,
    bvec (128,1) f32."""
    p, hh = core // 2, core % 2
    py = (hh * 128 + np.arange(128, dtype=np.float64)) / SIZE
    px = np.arange(SIZE, dtype=np.float64) / SIZE
    a = C[p]                          # (64, 2) float64
    b = np.roll(a, -1, axis=0)
    ay, ax = a[:, 0], a[:, 1]
    by, bx = b[:, 0], b[:, 1]

    lhsT = np.zeros((DROWS, NPAIR, 128), _BF)
    rhs = np.zeros((DROWS, NPAIR, 512), _BF)
    ones_j = np.ones(256, _BF)
    bsum = np.zeros(128, np.float64)

    for k in range(K):
        pp, t = k // 2, k % 2
        cs = slice(t * 256, (t + 1) * 256)
        v1x, v1y = bx[k] - ax[k], by[k] - ay[k]
        m_up = ((ay[k] <= py) & (py < by[k])).astype(np.float64)
        m_dn = ((by[k] <= py) & (py < ay[k])).astype(np.float64)
        mask = m_up + m_dn
        bsum -= m_dn

        PM = v1x * py + (v1y * ax[k] - v1x * ay[k])
        vM = -v1y * px
        PMh, PMm = _split2(PM)
        vMh, vMm = _split2(vM)
        mrows = [
            ((mask * PMh.astype(np.float64)).astype(_BF), ones_j),
            ((mask * PMm.astype(np.float64)).astype(_BF), ones_j),
            (mask.astype(_BF), vMh),
            (mask.astype(_BF), vMm),
            (((mask - 1.0) * 8.0).astype(_BF), ones_j),
        ]
        PQ = (py - ay[k]) ** 2
        vQ = (px - ax[k]) ** 2
        PQh, PQm = _split2(PQ)
        vQh, vQm = _split2(vQ)
        ones_i = np.ones(128, _BF)
        qrows = [
            (PQh, ones_j),
            (PQm, ones_j),
            (ones_i, vQh),
            (ones_i, vQm),
        ]
        for r, (li, rj) in enumerate(mrows):
            lhsT[t * 5 + r, pp, :] = li
            rhs[t * 5 + r, pp, cs] = rj
        for r, (li, rj) in enumerate(qrows):
            lhsT[MROWS + t * 4 + r, pp, :] = li
            rhs[MROWS + t * 4 + r, pp, cs] = rj

    bvec = (32.0 + bsum).astype(np.float32).reshape(128, 1)
    return {
        "lhsT": lhsT.reshape(DROWS, -1),
        "rhs": rhs.reshape(DROWS, -1),
        "bvec": bvec,
        "ident": np.eye(128, dtype=_BF),
    }


_PROGRAM = None


def _build_program():
    nc = bacc.Bacc("TRN2", target_bir_lowering=False, debug=False,
                   enable_asserts=False, num_devices=1)
    lhsT_d = nc.dram_tensor("lhsT", [DROWS, NPAIR * 128], BF16,
                            kind="ExternalInput").ap()
    rhs_d = nc.dram_tensor("rhs", [DROWS, NPAIR * 512], BF16,
                           kind="ExternalInput").ap()
    bvec_d = nc.dram_tensor("bvec", [128, 1], F32, kind="ExternalInput").ap()
    ident_d = nc.dram_tensor("ident", [128, 128], BF16,
                             kind="ExternalInput").ap()
    out_d = nc.dram_tensor("pm2", [128, SIZE], F32, kind="ExternalOutput").ap()

    AF = mybir.ActivationFunctionType
    ALU = mybir.AluOpType
    with tile.TileContext(nc, pool_alloc_mode="queue") as tc:
        with tc.tile_pool(name="lhsp", bufs=1) as lhsp, \
             tc.tile_pool(name="rhsp", bufs=4) as rhsp, \
             tc.tile_pool(name="sgp", bufs=2) as sgp, \
             tc.tile_pool(name="fin", bufs=1) as finp, \
             tc.tile_pool(name="mps", bufs=2, space="PSUM") as mps, \
             tc.tile_pool(name="qps", bufs=1, space="PSUM") as qps, \
             tc.tile_pool(name="wps", bufs=1, space="PSUM") as wps:

            lhsT_sb = lhsp.tile([40, NPAIR * 128], BF16)
            nc.sync.dma_start(lhsT_sb[0:MROWS, :], lhsT_d[0:MROWS, :])
            nc.sync.dma_start(lhsT_sb[32:32 + QROWS, :],
                              lhsT_d[MROWS:DROWS, :])
            ident_sb = lhsp.tile([128, 128], BF16)
            nc.gpsimd.dma_start(ident_sb[:, :], ident_d[:, :])
            bvec_sb = lhsp.tile([128, 1], F32)
            nc.gpsimd.dma_start(bvec_sb[:, :], bvec_d[:, :])

            minacc = finp.tile([128, 1024], F32)
            nc.vector.memset(minacc[:, :], MINACC_INIT)

            wacc = wps.tile([128, 1024], F32)
            sg_prev = None
            for g in range(NGRP):
                mt = mps.tile([128, 1024], F32, tag="mt")
                qt = qps.tile([128, 1024], F32, tag="qt")
                rhs_ts = []
                for h in range(2):
                    pp = g * 2 + h
                    rhs_t = rhsp.tile([40, 512], BF16, tag="rhs")
                    nc.sync.dma_start(rhs_t[0:MROWS, :],
                                      rhs_d[0:MROWS, pp * 512:(pp + 1) * 512])
                    nc.sync.dma_start(rhs_t[32:32 + QROWS, :],
                                      rhs_d[MROWS:DROWS,
                                            pp * 512:(pp + 1) * 512])
                    rhs_ts.append(rhs_t)
                # fold of the previous group's sign field rides between the
                # current group's matmuls (its sg input is already ready)
                if sg_prev is not None:
                    for q in range(2):
                        cs = slice(q * 512, (q + 1) * 512)
                        nc.tensor.matmul(wacc[:, cs], ident_sb[:, :],
                                         sg_prev[:, cs], start=(g == 1),
                                         stop=False, skip_group_check=True)
                for h in range(2):
                    pp = g * 2 + h
                    ls = slice(pp * 128, (pp + 1) * 128)
                    nc.tensor.matmul(mt[:, h * 512:(h + 1) * 512],
                                     lhsT_sb[0:MROWS, ls],
                                     rhs_ts[h][0:MROWS, :],
                                     start=True, stop=True)
                    nc.tensor.matmul(qt[:, h * 512:(h + 1) * 512],
                                     lhsT_sb[32:32 + QROWS, ls],
                                     rhs_ts[h][32:32 + QROWS, :],
                                     start=True, stop=True,
                                     tile_position=(32, 0))
                sg = sgp.tile([128, 1024], BF16, tag="sg")
                nc.scalar.activation(sg[:, :], mt[:, :], AF.Sign)
                nc.vector.tensor_tensor(minacc[:, :], minacc[:, :], qt[:, :],
                                        op=ALU.min)
                sg_prev = sg
            for q in range(2):
                cs = slice(q * 512, (q + 1) * 512)
                nc.tensor.matmul(wacc[:, cs], ident_sb[:, :], sg_prev[:, cs],
                                 start=False, stop=True,
                                 skip_group_check=True)

            # finals: W = 0.5*Σsign + B ; out = W² · min_k dist²
            ws0 = finp.tile([128, 512], F32)
            nc.scalar.copy(ws0[:, :], wacc[:, 0:512])
            ws1 = finp.tile([128, 512], F32)
            nc.vector.tensor_tensor(ws1[:, :], ws0[:, :],
                                    wacc[:, 512:1024], op=ALU.add)
            ws2 = finp.tile([128, 256], F32)
            nc.vector.tensor_tensor(ws2[:, :], ws1[:, 0:256], ws1[:, 256:512],
                                    op=ALU.add)
            wv = finp.tile([128, 256], F32)
            nc.scalar.activation(wv[:, :], ws2[:, :], AF.Identity,
                                 bias=bvec_sb[:, :], scale=0.5)
            wsq = finp.tile([128, 256], F32)
            nc.scalar.activation(wsq[:, :], wv[:, :], AF.Square)
            m1 = finp.tile([128, 512], F32)
            nc.vector.tensor_tensor(m1[:, :], minacc[:, 0:512],
                                    minacc[:, 512:1024], op=ALU.min)
            m2 = finp.tile([128, 256], F32)
            nc.vector.tensor_tensor(m2[:, :], m1[:, 0:256], m1[:, 256:512],
                                    op=ALU.min)
            outt = finp.tile([128, 256], F32)
            nc.vector.tensor_tensor(outt[:, :], wsq[:, :], m2[:, :],
                                    op=ALU.mult)
            nc.sync.dma_start(out_d[:, :], outt[:, :])

    nc.compile()
    return nc


def _get_program():
    global _PROGRAM
    if _PROGRAM is None:
        _PROGRAM = _build_program()
    return _PROGRAM


def _build_in_maps(C):
    return [_core_coeffs(C, core) for core in range(8)]


def kernel(contour: np.ndarray) -> np.ndarray:
    contour = np.asarray(contour)
    b, n, k, _ = contour.shape
    assert (b, n, k) == (2, 2, K)
    C = contour.reshape(b * n, K, 2).astype(np.float64)

    nc = _get_program()
    in_maps = _build_in_maps(C)
    res = bass_utils.run_bass_kernel_spmd(nc, in_maps, core_ids=list(range(8)))

    pm2 = np.stack([res.results[c]["pm2"] for c in range(8)])  # (8,128,256)
    pm = np.sqrt(np.maximum(pm2.astype(np.float64), 0.0))
    dmap = (pm / pm.max()).astype(np.float32)
    out = np.zeros((b * n, SIZE, SIZE), np.float32)
    for core in range(8):
        p, hh = core // 2, core % 2
        out[p, hh * 128:(hh + 1) * 128, :] = dmap[core]
    return out.reshape(b, n, SIZE, SIZE)


# revision 8
# speedup vs baseline: 2.2777x; 1.3259x over previous
"""Trainium2 Bass kernel for nn_Contour_to_distance_map.

Reformulation: the reference winding |Σ_k tanh(1e5·cross_k)·arccos(clip(cos_k))|/2π
equals the integer ray-crossing count for all pixels outside hair-thin bands
around edge lines (validated: rel L2 vs reference ~9e-3 << 2e-2 budget).

Per pixel m=(py,px) (py=coord0=partition/row, px=coord1=col) and edge a_k→b_k:
  crossing contribution f_k = m_up·[cL>0] − m_dn·[cL<0]
    cL_k(i,j) = v1x·py_i − v1y·px_j + (v1y·ax − v1x·ay)
    m_up = [ay ≤ py < by], m_dn = [by ≤ py < ay]  (per-partition constants)
  w = Σ_k f_k = 0.5·Σ_k sign(M_k) + B(part)
    M_k = mask_k(i)·cL_k − 8·(1−mask_k(i))   (strictly negative when masked out)
    B(part) = 32 − Σ_k m_dn_k(part)
  out = |w| · min_k dist(m, a_k); device computes w²·min_k dist²; host sqrts
  and applies the global max normalization (scale-invariant).

All per-k fields are outer sums P_k(i)+v_k(j) → tiny-contraction bf16-2-split
matmuls. PE produces M and Q1=dist² psum tiles and accumulates the sign-field
sums via identity matmuls (exact: signs are ±1 bf16, psum fp32); ACT does Sign
(psum→sbuf bf16; sign is in every ACT table set → no table thrash); DVE does
the running min over Q1; GpSimd pre-reduces sign fields pairwise so the PE
fold count halves. All lhsT/rhs coefficients are prefetched in a few large
DMAs. Data-parallel: core c → polygon c//2, row-half c%2.
"""

import numpy as np
import ml_dtypes

import concourse.bass as bass
import concourse.bacc as bacc
import concourse.tile as tile
import concourse.mybir as mybir
import concourse.bass_utils as bass_utils

F32 = mybir.dt.float32
BF16 = mybir.dt.bfloat16

SIZE = 256
K = 64
NPAIR = K // 2           # 32 pairs of vertices, 2 k per pair
NGRP = NPAIR // 2        # 16 groups of 2 pairs (4 k)
# lhsT/rhs row layout (compact DRAM form, 18 rows):
#   rows [0:10)  M block: per k 5 rows: maskPMh, maskPMm, mask(vMh), mask(vMm),
#                -8(1-mask)
#   rows [10:18) Q block: per k 4 rows: PQh, PQm, vQh, vQm
# SBUF form is [40, ...]: M at partitions 0:10, Q at partitions 32:40 so the
# two matmuls sit in different PE row groups.
MROWS = 10
QROWS = 8
DROWS = MROWS + QROWS
MINACC_INIT = 3.0e38
GPSIMD_PREREDUCE = True

_BF = ml_dtypes.bfloat16


def _split2(x):
    """f64 -> two bf16 planes summing to ~16-bit-mantissa precision."""
    h = np.asarray(x, _BF).astype(np.float64)
    m = np.asarray(x - h, _BF).astype(np.float64)
    return h.astype(_BF), m.astype(_BF)


def _core_coeffs(C, core):
    """Inputs for one core: lhsT (18, NPAIR*128), rhs (18, NPAIR*512) bf16,
    bvec (128,1) f32, ident (128,128) bf16."""
    p, hh = core // 2, core % 2
    py = (hh * 128 + np.arange(128, dtype=np.float64)) / SIZE
    px = np.arange(SIZE, dtype=np.float64) / SIZE
    a = C[p]                          # (64, 2) float64
    b = np.roll(a, -1, axis=0)
    ay, ax = a[:, 0], a[:, 1]
    by, bx = b[:, 0], b[:, 1]

    lhsT = np.zeros((DROWS, NPAIR, 128), _BF)
    rhs = np.zeros((DROWS, NPAIR, 512), _BF)
    ones_j = np.ones(256, _BF)
    bsum = np.zeros(128, np.float64)

    for k in range(K):
        pp, t = k // 2, k % 2
        cs = slice(t * 256, (t + 1) * 256)
        v1x, v1y = bx[k] - ax[k], by[k] - ay[k]
        m_up = ((ay[k] <= py) & (py < by[k])).astype(np.float64)
        m_dn = ((by[k] <= py) & (py < ay[k])).astype(np.float64)
        mask = m_up + m_dn
        bsum -= m_dn

        PM = v1x * py + (v1y * ax[k] - v1x * ay[k])
        vM = -v1y * px
        PMh, PMm = _split2(PM)
        vMh, vMm = _split2(vM)
        mrows = [
            ((mask * PMh.astype(np.float64)).astype(_BF), ones_j),
            ((mask * PMm.astype(np.float64)).astype(_BF), ones_j),
            (mask.astype(_BF), vMh),
            (mask.astype(_BF), vMm),
            (((mask - 1.0) * 8.0).astype(_BF), ones_j),
        ]
        PQ = (py - ay[k]) ** 2
        vQ = (px - ax[k]) ** 2
        PQh, PQm = _split2(PQ)
        vQh, vQm = _split2(vQ)
        ones_i = np.ones(128, _BF)
        qrows = [
            (PQh, ones_j),
            (PQm, ones_j),
            (ones_i, vQh),
            (ones_i, vQm),
        ]
        for r, (li, rj) in enumerate(mrows):
            lhsT[t * 5 + r, pp, :] = li
            rhs[t * 5 + r, pp, cs] = rj
        for r, (li, rj) in enumerate(qrows):
            lhsT[MROWS + t * 4 + r, pp, :] = li
            rhs[MROWS + t * 4 + r, pp, cs] = rj

    bvec = (32.0 + bsum).astype(np.float32).reshape(128, 1)
    return {
        "lhsT": lhsT.reshape(DROWS, -1),
        "rhs": rhs.reshape(DROWS, -1),
        "bvec": bvec,
        "ident": np.eye(128, dtype=_BF),
    }


_PROGRAM = None


def _build_program():
    nc = bacc.Bacc("TRN2", target_bir_lowering=False, debug=False,
                   enable_asserts=False, num_devices=1)
    lhsT_d = nc.dram_tensor("lhsT", [DROWS, NPAIR * 128], BF16,
                            kind="ExternalInput").ap()
    rhs_d = nc.dram_tensor("rhs", [DROWS, NPAIR * 512], BF16,
                           kind="ExternalInput").ap()
    bvec_d = nc.dram_tensor("bvec", [128, 1], F32, kind="ExternalInput").ap()
    ident_d = nc.dram_tensor("ident", [128, 128], BF16,
                             kind="ExternalInput").ap()
    out_d = nc.dram_tensor("pm2", [128, SIZE], F32, kind="ExternalOutput").ap()

    AF = mybir.ActivationFunctionType
    ALU = mybir.AluOpType
    RW = NPAIR * 512
    with tile.TileContext(nc, pool_alloc_mode="queue") as tc:
        with tc.tile_pool(name="lhsp", bufs=1) as lhsp, \
             tc.tile_pool(name="sgp", bufs=3) as sgp, \
             tc.tile_pool(name="sggp", bufs=2) as sggp, \
             tc.tile_pool(name="fin", bufs=1) as finp, \
             tc.tile_pool(name="mps", bufs=2, space="PSUM") as mps, \
             tc.tile_pool(name="qps", bufs=2, space="PSUM") as qps, \
             tc.tile_pool(name="wps", bufs=1, space="PSUM") as wps:

            # ---- prefetch all coefficients in a few large DMAs ----
            lhsT_sb = lhsp.tile([40, NPAIR * 128], BF16)
            nc.sync.dma_start(lhsT_sb[0:MROWS, :], lhsT_d[0:MROWS, :])
            nc.gpsimd.dma_start(lhsT_sb[32:32 + QROWS, :],
                                lhsT_d[MROWS:DROWS, :])
            rhs_sb = lhsp.tile([40, RW], BF16)
            # chunked so the first matmuls can start before the tail arrives
            NCH = 4
            cw = RW // NCH
            for c in range(NCH):
                cs = slice(c * cw, (c + 1) * cw)
                nc.sync.dma_start(rhs_sb[0:MROWS, cs], rhs_d[0:MROWS, cs])
                nc.gpsimd.dma_start(rhs_sb[32:32 + QROWS, cs],
                                    rhs_d[MROWS:DROWS, cs])
            ident_sb = lhsp.tile([128, 128], BF16)
            nc.gpsimd.dma_start(ident_sb[:, :], ident_d[:, :])
            bvec_sb = lhsp.tile([128, 1], F32)
            nc.gpsimd.dma_start(bvec_sb[:, :], bvec_d[:, :])

            minacc = finp.tile([128, 1024], F32)
            nc.vector.memset(minacc[:, :], MINACC_INIT)

            wacc = wps.tile([128, 512], F32)
            sgs = []      # per-pair sign tiles awaiting fold
            folded = 0    # number of fold matmuls issued
            total_folds = NGRP if GPSIMD_PREREDUCE else NPAIR

            def fold(rhs_ap):
                nonlocal folded
                nc.tensor.matmul(wacc[:, :], ident_sb[:, :], rhs_ap,
                                 start=(folded == 0),
                                 stop=(folded == total_folds - 1),
                                 skip_group_check=True)
                folded += 1

            for g in range(NGRP):
                qt = qps.tile([128, 1024], F32, tag="qt")
                for h in range(2):
                    pp = g * 2 + h
                    ls = slice(pp * 128, (pp + 1) * 128)
                    rs = slice(pp * 512, (pp + 1) * 512)
                    mt = mps.tile([128, 512], F32, tag="mt")
                    nc.tensor.matmul(mt[:, :], lhsT_sb[0:MROWS, ls],
                                     rhs_sb[0:MROWS, rs],
                                     start=True, stop=True)
                    nc.tensor.matmul(qt[:, h * 512:(h + 1) * 512],
                                     lhsT_sb[32:32 + QROWS, ls],
                                     rhs_sb[32:32 + QROWS, rs],
                                     start=True, stop=True,
                                     tile_position=(32, 0))
                    sg = sgp.tile([128, 512], BF16, tag="sg")
                    nc.scalar.activation(sg[:, :], mt[:, :], AF.Sign)
                    sgs.append(sg)
                    if len(sgs) == 2:
                        if GPSIMD_PREREDUCE:
                            sgg = sggp.tile([128, 512], BF16, tag="sgg")
                            nc.gpsimd.tensor_tensor(sgg[:, :], sgs[0][:, :],
                                                    sgs[1][:, :], op=ALU.add)
                            fold(sgg[:, :])
                        else:
                            fold(sgs[0][:, :])
                            fold(sgs[1][:, :])
                        sgs = []
                nc.vector.tensor_tensor(minacc[:, :], minacc[:, :], qt[:, :],
                                        op=ALU.min)
            for sg in sgs:
                fold(sg[:, :])
            assert folded == total_folds

            # finals: W = 0.5*Σsign + B ; out = W² · min_k dist²
            ws0 = finp.tile([128, 512], F32)
            nc.scalar.copy(ws0[:, :], wacc[:, :])
            ws2 = finp.tile([128, 256], F32)
            nc.vector.tensor_tensor(ws2[:, :], ws0[:, 0:256], ws0[:, 256:512],
                                    op=ALU.add)
            wv = finp.tile([128, 256], F32)
            nc.scalar.activation(wv[:, :], ws2[:, :], AF.Identity,
                                 bias=bvec_sb[:, :], scale=0.5)
            wsq = finp.tile([128, 256], F32)
            nc.scalar.activation(wsq[:, :], wv[:, :], AF.Square)
            m1 = finp.tile([128, 512], F32)
            nc.vector.tensor_tensor(m1[:, :], minacc[:, 0:512],
                                    minacc[:, 512:1024], op=ALU.min)
            m2 = finp.tile([128, 256], F32)
            nc.vector.tensor_tensor(m2[:, :], m1[:, 0:256], m1[:, 256:512],
                                    op=ALU.min)
            outt = finp.tile([128, 256], F32)
            nc.vector.tensor_tensor(outt[:, :], wsq[:, :], m2[:, :],
                                    op=ALU.mult)
            nc.sync.dma_start(out_d[:, :], outt[:, :])

    nc.compile()
    return nc


def _get_program():
    global _PROGRAM
    if _PROGRAM is None:
        _PROGRAM = _build_program()
    return _PROGRAM


def _build_in_maps(C):
    return [_core_coeffs(C, core) for core in range(8)]


def kernel(contour: np.ndarray) -> np.ndarray:
    contour = np.asarray(contour)
    b, n, k, _ = contour.shape
    assert (b, n, k) == (2, 2, K)
    C = contour.reshape(b * n, K, 2).astype(np.float64)

    nc = _get_program()
    in_maps = _build_in_maps(C)
    res = bass_utils.run_bass_kernel_spmd(nc, in_maps, core_ids=list(range(8)))

    pm2 = np.stack([res.results[c]["pm2"] for c in range(8)])  # (8,128,256)
    pm = np.sqrt(np.maximum(pm2.astype(np.float64), 0.0))
    dmap = (pm / pm.max()).astype(np.float32)
    out = np.zeros((b * n, SIZE, SIZE), np.float32)
    for core in range(8):
        p, hh = core // 2, core % 2
        out[p, hh * 128:(hh + 1) * 128, :] = dmap[core]
    return out.reshape(b, n, SIZE, SIZE)
